# revision 1
# baseline (speedup 1.0000x reference)
"""GCN classifier on 8 TRN2 NeuronCores.

Row-shards the 16384-node graph across 8 cores (2048 rows each). All
activations stay feature-major ([feat, nodes]) on chip so every matmul
contracts over the partition dimension. Pass 1 streams the 1 GiB fp32
adjacency once with 1-MiB DMAs split across both HWDGE rings: casts to bf16,
transposes 128x128 blocks on the TensorEngine, computes row degrees with
ones-matmuls over the transposed tiles, and writes A^T to DRAM as fp8-e4m3
in 8-MiB batched SWDGE writes. The two GCN aggregation passes then stream
A^T back (4 j-blocks per DMA, alternating SP/ACT rings) into fp32-PSUM
matmuls against AllGathered bf16 features; dis-scaling, bias and relu run
fused on DVE/ACT out of PSUM.

Collective schedule: the UNSCALED XW1 is AllGathered right after the
encoder, fully hidden behind pass 1's streaming; the degree-derived dis
vector (64 KiB) is AllGathered at the end of pass 1; pass 2 then applies
dis_j to gathered XW1 tiles on the fly (ACT is idle there). Same for XW2 /
pass 3. This removes the two large post-pass AllGather stalls.

Self-contained: hardcodes shapes for nn_GCNClassifer_6786048328674
(relative error vs the fp64 reference ~7e-4, gate 2e-2).
"""

import sys

sys.path.insert(0, "/opt/trn_rl_repo")

from contextlib import ExitStack

import numpy as np

import concourse.bass as bass
from concourse import bacc
import concourse.mybir as mybir
from concourse.tile import TileContext, add_dep_helper
from concourse.bass_utils import run_bass_kernel_spmd
from concourse.masks import make_identity

F32 = mybir.dt.float32
BF16 = mybir.dt.bfloat16
AF = mybir.ActivationFunctionType
ALU = mybir.AluOpType

BN_EPS = 1e-5
N_CORES = 8
P = 128
A_DT = "fp8"           # "bf16" | "fp8" storage dtype for the cached A^T
STOP_AFTER_PASS1 = False


def build_nc(N=16384, F=1024, D1=512, E=256, H=256, G=128, C=10, n_cores=N_CORES):
    R = N // n_cores            # rows (nodes) per core
    assert R % 512 == 0 and N % 2048 == 0
    IB = R // P                 # 128-row blocks per core
    JW = 512                    # pass-1 column chunk width
    JC = N // JW                # pass-1 column chunks
    JO = N // P                 # 128-wide j blocks (passes 2/3)
    JB = 4                      # j-blocks batched per DMA in passes 2/3
    IC = R // 512               # 512-wide i chunks
    ADT = BF16 if A_DT == "bf16" else mybir.dt.float8e4

    nc = bacc.Bacc(num_devices=n_cores)

    # ---- I/O -------------------------------------------------------------
    a_d = nc.declare_dram_parameter("a", [R, N], F32, isOutput=False)
    xT_d = nc.declare_dram_parameter("xT", [F, R], F32, isOutput=False)
    w1_d = nc.declare_dram_parameter("w1", [F, D1], F32, isOutput=False)
    k1_d = nc.declare_dram_parameter("k1", [D1], F32, isOutput=False)
    c1_d = nc.declare_dram_parameter("c1", [D1], F32, isOutput=False)
    w2_d = nc.declare_dram_parameter("w2", [D1, E], F32, isOutput=False)
    k2_d = nc.declare_dram_parameter("k2", [E], F32, isOutput=False)
    c2_d = nc.declare_dram_parameter("c2", [E], F32, isOutput=False)
    g1w_d = nc.declare_dram_parameter("g1w", [E, H], F32, isOutput=False)
    g1b_d = nc.declare_dram_parameter("g1b", [H], F32, isOutput=False)
    g2w_d = nc.declare_dram_parameter("g2w", [H, G], F32, isOutput=False)
    g2b_d = nc.declare_dram_parameter("g2b", [G], F32, isOutput=False)
    cw_d = nc.declare_dram_parameter("cw", [G, C], F32, isOutput=False)
    cb_d = nc.declare_dram_parameter("cb", [C], F32, isOutput=False)
    out_d = nc.declare_dram_parameter("out", [C, R], F32, isOutput=True)

    # ---- collective DRAM tensors ----------------------------------------
    # NOTE: declare the small Shared tensor FIRST -- a small Shared tensor
    # allocated after large ones produces a NEFF the runtime fails to load.
    dis_loc = nc.dram_tensor("dis_loc", [1, R], F32)
    DISg = nc.dram_tensor("DISg", [n_cores, R], F32, addr_space="Shared")
    xw1_loc = nc.dram_tensor("xw1_loc", [R, H], BF16)
    XW1g = nc.dram_tensor("XW1g", [N, H], BF16, addr_space="Shared")
    xw2a_loc = nc.dram_tensor("xw2a_loc", [R // 2, G], BF16)
    XW2gA = nc.dram_tensor("XW2gA", [N // 2, G], BF16, addr_space="Shared")
    xw2b_loc = nc.dram_tensor("xw2b_loc", [R // 2, G], BF16)
    XW2gB = nc.dram_tensor("XW2gB", [N // 2, G], BF16, addr_space="Shared")
    groups = [list(range(n_cores))]

    with TileContext(nc) as tc, ExitStack() as ctx:
        wpool = ctx.enter_context(tc.tile_pool(name="wpool", bufs=1))
        dram = ctx.enter_context(tc.tile_pool(name="dram", bufs=1, space="DRAM"))
        io_pool = ctx.enter_context(tc.tile_pool(name="io", bufs=3))
        io2_pool = ctx.enter_context(tc.tile_pool(name="io2", bufs=2))
        psum = ctx.enter_context(tc.tile_pool(name="psum", bufs=1, space="PSUM"))
        _psn = [0]

        def ps_tile(shape, dtype, banks=range(8), name=None):
            tag = "b%d" % (list(banks)[_psn[0] % len(list(banks))])
            _psn[0] += 1
            return psum.tile(shape, dtype, tag=tag, name=name or f"ps{_psn[0]}")

        # ---- constants / weights in SBUF --------------------------------
        idb = wpool.tile([P, P], BF16)
        make_identity(nc, idb)
        idf = wpool.tile([P, P], F32)
        make_identity(nc, idf)
        ones_b = wpool.tile([P, 1], BF16)
        nc.vector.memset(ones_b, 1.0)

        def load_vec(d, n, nm):
            t = wpool.tile([P, n // P], F32, tag=nm, name=nm)
            nc.gpsimd.dma_start(t, d.ap().rearrange("(o p) -> p o", p=P))
            return t

        # only what encoder strip 0's first stage needs, so its x-strip and
        # first matmuls start ~10us earlier; the rest loads behind it
        w1_sb = wpool.tile([P, F // P, D1], BF16)
        nc.gpsimd.dma_start(w1_sb, w1_d.ap().rearrange("(ko p) m -> p ko m", p=P))
        k1_sb = load_vec(k1_d, D1, "k1v")
        c1_sb = load_vec(c1_d, D1, "c1v")

        def load_rest():
            out = {}
            out["w2_sb"] = wpool.tile([P, D1 // P, E], BF16, name="w2_sb")
            nc.gpsimd.dma_start(out["w2_sb"],
                                w2_d.ap().rearrange("(ko p) m -> p ko m", p=P))
            out["g1w_sb"] = wpool.tile([P, E // P, H], F32, name="g1w_sb")
            nc.gpsimd.dma_start(out["g1w_sb"],
                                g1w_d.ap().rearrange("(ko p) m -> p ko m", p=P))
            out["g2w_sb"] = wpool.tile([P, H // P, G], F32, name="g2w_sb")
            nc.gpsimd.dma_start(out["g2w_sb"],
                                g2w_d.ap().rearrange("(ko p) m -> p ko m", p=P))
            out["cw_sb"] = wpool.tile([G, C], F32, name="cw_sb")
            nc.gpsimd.dma_start(out["cw_sb"], cw_d[:, :])
            out["k2_sb"] = load_vec(k2_d, E, "k2v")
            out["c2_sb"] = load_vec(c2_d, E, "c2v")
            out["g1b_sb"] = load_vec(g1b_d, H, "g1bv")
            out["g2b_sb"] = load_vec(g2b_d, G, "g2bv")
            out["cb_sb"] = wpool.tile([C, 1], F32, name="cb_sb")
            nc.gpsimd.dma_start(out["cb_sb"],
                                cb_d.ap().rearrange("(c o) -> c o", o=1))
            return out

        # ---- persistent activations (feature-major) ----------------------
        h1_sb = wpool.tile([P, D1 // P, R], BF16, tag="tagA")  # 2 MiB
        h2_sb = wpool.tile([P, E // P, R], F32, tag="tagB")    # 2 MiB
        xw1_sb = wpool.tile([P, H // P, R], BF16, tag="tagC")  # 1 MiB
        out_sb = wpool.tile([C, R], F32)
        dis_bc = wpool.tile([P, R], F32)                       # dis broadcast, 1 MiB
        dis_all = wpool.tile([P, JO], F32)                     # dis_j all nodes, 64 KiB

        # ===== encoder strips (interleaved with pass-1 groups below) =====
        xT_r = xT_d.ap().rearrange("(ko p) i -> p ko i", p=P)

        def enc_strip(s):
            isl = bass.ts(s, 512)
            xs = io2_pool.tile([P, F // P, 512], BF16, tag="wide", bufs=3,
                               name=f"xs{s}")
            nc.gpsimd.dma_start(xs, xT_r[:, :, isl])
            if s == 0:
                _late.update(load_rest())
            for m in range(D1 // P):
                ps = ps_tile([P, 512], F32, banks=range(4))
                for k in range(F // P):
                    nc.tensor.matmul(ps, w1_sb[:, k, bass.ts(m, P)], xs[:, k],
                                     start=(k == 0), stop=(k == F // P - 1))
                nc.scalar.activation(h1_sb[:, m, isl], ps, AF.Relu,
                                     bias=c1_sb[:, m:m + 1],
                                     scale=k1_sb[:, m:m + 1])
            for m in range(E // P):
                ps = ps_tile([P, 512], F32, banks=range(4))
                for k in range(D1 // P):
                    nc.tensor.matmul(ps, _late["w2_sb"][:, k, bass.ts(m, P)],
                                     h1_sb[:, k, isl],
                                     start=(k == 0), stop=(k == D1 // P - 1))
                nc.scalar.activation(h2_sb[:, m, isl], ps, AF.Relu,
                                     bias=_late["c2_sb"][:, m:m + 1],
                                     scale=_late["k2_sb"][:, m:m + 1])
            for m in range(H // P):
                ps = ps_tile([P, 512], F32, banks=range(4))
                for k in range(E // P):
                    nc.tensor.matmul(ps, _late["g1w_sb"][:, k, bass.ts(m, P)],
                                     h2_sb[:, k, isl],
                                     start=(k == 0), stop=(k == E // P - 1))
                nc.vector.tensor_copy(xw1_sb[:, m, isl], ps)

        # =========== pass 1: stream A, cast, PE-transpose, col-degs =======
        a_q = dram.tile([N, R], ADT)
        a_q_w = a_q.rearrange("(g t p) i -> p (g t) i", p=P, t=JW // P)
        dps_row = [psum.tile([1, 512], F32, tag=f"b{4 + i}", name=f"degps{i}")
                   for i in range(IC)]
        JG = 1024                     # columns per group
        NT = JG // P                  # 8 transposed blocks per group
        NJG = N // JG
        _p1 = {}

        def p1_group(jg):
            wide = io2_pool.tile([P, NT, R], ADT, tag="wide", bufs=3,
                                 name=f"wide{jg}")
            for ib in range(IB):
                at = io2_pool.tile([P, JG], F32, tag="a_in", bufs=6,
                                   name=f"at{jg}_{ib}")
                eng_d = nc.sync if ib % 2 == 0 else nc.scalar
                da = eng_d.dma_start(at, a_d[bass.ts(ib, P), bass.ts(jg, JG)])
                if ib % 2 == 1:
                    _p1["last_at_act"] = da
                ab = io2_pool.tile([P, JG], BF16, tag="a_cast",
                                   name=f"ab{jg}_{ib}")
                if ib % 2 == 0:
                    nc.scalar.activation(ab, at, AF.Copy)
                else:
                    nc.vector.tensor_copy(ab, at)
                for half in range(2):
                    pst = ps_tile([P, JG // 2], BF16, banks=range(4))
                    for t in range(NT // 2):
                        tt_ = half * (NT // 2) + t
                        nc.tensor.transpose(pst[:, bass.ts(t, P)],
                                            ab[:, bass.ts(tt_, P)], idb)
                    dst = wide[:, half * (NT // 2):(half + 1) * (NT // 2),
                               bass.ts(ib, P)]
                    if (ib + half) % 2 == 0:
                        nc.vector.tensor_copy(
                            dst, pst.rearrange("p (t i) -> p t i", t=NT // 2))
                    else:
                        nc.scalar.activation(
                            dst, pst.rearrange("p (t i) -> p t i", t=NT // 2),
                            AF.Copy)
            for t in range(NT):
                for i in range(IC):
                    nc.tensor.matmul(
                        dps_row[i], ones_b, wide[:, t, bass.ts(i, 512)],
                        start=(jg == 0 and t == 0),
                        stop=(jg == NJG - 1 and t == NT - 1))
            _p1["wq_last"] = nc.gpsimd.dma_start(
                a_q_w[:, bass.ts(jg, NT), :], wide)

        # interleave: each encoder strip is followed by 3 pass-1 groups so
        # the PE queue (and PSUM banks) alternate and A-streaming starts at
        # t=0 instead of waiting out the whole encoder
        _late = {}
        for s in range(IC):
            enc_strip(s)
            p1_group(s)
        g2w_sb, cw_sb = _late["g2w_sb"], _late["cw_sb"]
        g1b_sb, g2b_sb, cb_sb = (_late["g1b_sb"], _late["g2b_sb"],
                                 _late["cb_sb"])

        # ======= xw1 -> natural layout -> AllGather (hidden by pass 1) ====
        for it in range(IB):
            pst = ps_tile([P, H], BF16, banks=range(4))
            for m in range(H // P):
                nc.tensor.transpose(pst[:, bass.ts(m, P)],
                                    xw1_sb[:, m, bass.ts(it, P)], idb)
            snat = io_pool.tile([P, H], BF16, tag="s1nat")
            nc.scalar.activation(snat, pst, AF.Copy)
            nc.gpsimd.dma_start(xw1_loc[bass.ts(it, P), :], snat)
        cc1 = nc.gpsimd.collective_compute(
            "AllGather", ALU.bypass, replica_groups=groups,
            ins=[xw1_loc[:, :].opt()], outs=[XW1g[:, :].opt()])

        for jg in range(IC, NJG):
            p1_group(jg)
        wq_last = _p1["wq_last"]
        last_at_act = _p1["last_at_act"]

        # own-row degrees -> dis (broadcast + AllGather of the tiny vector)
        dmy = wpool.tile([1, R], F32)
        for i in range(IC):
            nc.vector.tensor_copy(dmy[:, bass.ts(i, 512)], dps_row[i])
        nc.vector.reciprocal(dmy, dmy)
        nc.scalar.activation(dmy, dmy, AF.Sqrt)
        nc.sync.dma_start(dis_loc[0:1, :], dmy)
        nc.sync.dma_start(dis_bc, dis_loc[0:1, :].to_broadcast([P, R]))
        ccd = nc.gpsimd.collective_compute(
            "AllGather", ALU.bypass, replica_groups=groups,
            ins=[dis_loc[:, :].opt()], outs=[DISg[:, :].opt()])
        # load gathered dis j-major [o, p] (contiguous rows), then
        # PE-transpose to [p, o]
        dis_nat = io_pool.tile([P, JO], F32, tag="disn")
        dd = nc.scalar.dma_start(
            dis_nat, DISg.ap().rearrange("c (o2 pb) -> (c o2) pb", pb=P))
        add_dep_helper(dd.ins, ccd.ins, reason="dis read after AG")
        ps_dis = ps_tile([P, JO], F32, banks=range(4, 8))
        nc.tensor.transpose(ps_dis, dis_nat, idf)
        nc.vector.tensor_copy(dis_all, ps_dis)

        if STOP_AFTER_PASS1:
            nc.vector.tensor_copy(out_sb, dis_bc[0:C, :])
            nc.sync.dma_start(out_d[:, :], out_sb)
        else:
            # ======= pass 2 (two i-rounds): y1 = (A @ S1)^T =============
            NB1 = E // P
            RH = R // 2
            JA = 2 * JB
            NJP = JO // JA
            a_q_r = a_q.rearrange("(o p) i -> p o i", p=P)
            S1_r = XW1g.ap().rearrange("(o p) n -> p o n", p=P)
            S2A_r = XW2gA.ap().rearrange("(o p) n -> p o n", p=P)
            S2B_r = XW2gB.ap().rearrange("(o p) n -> p o n", p=P)
            h3_sb = wpool.tile([P, H // P, R], F32, tag="tagB")
            xw2T_sb = wpool.tile([P, R], BF16)
            r1_pf = []
            cc2s = []
            for r in range(2):
                i0 = 2 * r
                ps_y = [psum.tile([P, 512], F32, tag=f"b{4 * r + m * 2 + ii}",
                                  name=f"ps_y{r}_{m}_{ii}")
                        for m in range(NB1) for ii in range(2)]
                for jp in range(NJP):
                    att = io2_pool.tile([P, JA, RH], ADT, tag="att2", bufs=3)
                    nc.sync.dma_start(
                        att, a_q_r[:, jp * JA:(jp + 1) * JA,
                                   r * RH:(r + 1) * RH])
                    for h in range(2):
                        jb0 = jp * JA + h * JB
                        if r == 1 and jp == 0:
                            s1t = r1_pf[h]
                        else:
                            s1t = io_pool.tile([P, JB, E], BF16, tag="sjo")
                            d2 = nc.scalar.dma_start(
                                s1t, S1_r[:, jb0:jb0 + JB, :])
                            add_dep_helper(d2.ins, cc1.ins,
                                           reason="S1 read after AG")
                            if r == 0 and jp == 0 and h == 0:
                                add_dep_helper(d2.ins, last_at_act.ins,
                                               reason="ACT ring order")
                        s1s = io_pool.tile([P, JB, E], BF16, tag="sjs")
                        for q in range(JB):
                            nc.vector.tensor_scalar_mul(
                                s1s[:, q], s1t[:, q],
                                dis_all[:, jb0 + q:jb0 + q + 1])
                        for q in range(JB):
                            jo = jb0 + q
                            for m in range(NB1):
                                for ii in range(2):
                                    nc.tensor.matmul(
                                        ps_y[m * 2 + ii],
                                        s1s[:, q, bass.ts(m, P)],
                                        att[:, h * JB + q, bass.ts(ii, 512)],
                                        start=(jo == 0), stop=(jo == JO - 1))
                if r == 0:
                    # prefetch round-1 jp0's S1 tiles so the ACT ring isn't
                    # blocked behind the xw2a write chain at the boundary
                    for h in range(2):
                        jb0 = h * JB
                        s1t = io_pool.tile([P, JB, E], BF16, tag="sjo")
                        d2 = nc.scalar.dma_start(s1t, S1_r[:, jb0:jb0 + JB, :])
                        add_dep_helper(d2.ins, cc1.ins,
                                       reason="S1 read after AG")
                        r1_pf.append(s1t)
                # epilogue: h3 = relu(dis * y1 + b) for this round's i-chunks
                for m in range(NB1):
                    for ii in range(2):
                        isl = bass.ts(i0 + ii, 512)
                        tt = io_pool.tile([P, 512], F32, tag="ep")
                        nc.vector.tensor_tensor(tt, ps_y[m * 2 + ii],
                                                dis_bc[:, isl], ALU.mult)
                        nc.scalar.activation(h3_sb[:, m, isl], tt, AF.Relu,
                                             bias=g1b_sb[:, m:m + 1])
                # xw2 for this round -> natural -> chunked AllGather
                for ii in range(2):
                    isl = bass.ts(i0 + ii, 512)
                    ps = ps_tile([P, 512], F32, banks=range(0, 4))
                    for k in range(H // P):
                        nc.tensor.matmul(ps, g2w_sb[:, k, :], h3_sb[:, k, isl],
                                         start=(k == 0),
                                         stop=(k == H // P - 1))
                    nc.vector.tensor_copy(xw2T_sb[:, isl], ps)
                xw2_loc_r = xw2a_loc if r == 0 else xw2b_loc
                for itl in range(IB // 2):
                    it = r * (IB // 2) + itl
                    pst = ps_tile([P, G], BF16, banks=range(0, 4))
                    nc.tensor.transpose(pst, xw2T_sb[:, bass.ts(it, P)], idb)
                    snat = io_pool.tile([P, G], BF16, tag="s2nat")
                    nc.scalar.activation(snat, pst, AF.Copy)
                    nc.gpsimd.dma_start(xw2_loc_r[bass.ts(itl, P), :], snat)
                if r == 1:
                    # prefetch pass-3 phase-0 A^T chunks before cc2b so their
                    # transfers overlap the AllGather latency
                    att3_pf = []
                    for c in range(3):
                        att = io2_pool.tile([P, JA, R], ADT, tag="wide",
                                            bufs=3)
                        nc.sync.dma_start(
                            att, a_q_r[:, 2 * c * JA:(2 * c + 1) * JA, :])
                        att3_pf.append(att)
                cc2s.append(nc.gpsimd.collective_compute(
                    "AllGather", ALU.bypass, replica_groups=groups,
                    ins=[xw2_loc_r[:, :].opt()],
                    outs=[(XW2gA if r == 0 else XW2gB)[:, :].opt()]))

            # ======= pass 3 (two phases): y2 = (A @ S2)^T =================
            ps_z = [psum.tile([P, 512], F32, tag=f"b{i}", name=f"ps_z_{i}")
                    for i in range(IC)]
            first = [True]
            for phase in range(2):
                S2_r = S2A_r if phase == 0 else S2B_r
                cc2p = cc2s[phase]
                for c in range(n_cores):
                    jp = 2 * c + phase
                    if phase == 0 and c < 3:
                        att = att3_pf[c]
                    else:
                        att = io2_pool.tile([P, JA, R], ADT, tag="wide",
                                            bufs=3)
                        nc.sync.dma_start(
                            att, a_q_r[:, jp * JA:(jp + 1) * JA, :])
                    for h in range(2):
                        jb0 = jp * JA + h * JB      # global j-block base
                        ob = c * JA + h * JB        # block within XW2gA/B
                        s2t = io_pool.tile([P, JB, G], BF16, tag="sjo")
                        d2 = nc.scalar.dma_start(s2t, S2_r[:, ob:ob + JB, :])
                        add_dep_helper(d2.ins, cc2p.ins,
                                       reason="S2 read after AG")
                        s2s = io_pool.tile([P, JB, G], BF16, tag="sjs")
                        for q in range(JB):
                            nc.vector.tensor_scalar_mul(
                                s2s[:, q], s2t[:, q],
                                dis_all[:, jb0 + q:jb0 + q + 1])
                        for q in range(JB):
                            last = (phase == 1 and c == n_cores - 1
                                    and h == 1 and q == JB - 1)
                            for i in range(IC):
                                nc.tensor.matmul(
                                    ps_z[i], s2s[:, q, :],
                                    att[:, h * JB + q, bass.ts(i, 512)],
                                    start=first[0], stop=last)
                            first[0] = False
            # ==== fused tail: per-chunk epilogue -> classifier -> write ===
            h4_sb = wpool.tile([P, R], F32, tag="tagC")
            for i in range(IC):
                isl = bass.ts(i, 512)
                tt = io_pool.tile([P, 512], F32, tag="ep")
                nc.vector.tensor_tensor(tt, ps_z[i], dis_bc[:, isl], ALU.mult)
                nc.scalar.activation(h4_sb[:, isl], tt, AF.Relu,
                                     bias=g2b_sb[:, 0:1])
                ps = ps_tile([C, 512], F32, banks=range(0, 4))
                nc.tensor.matmul(ps, cw_sb, h4_sb[:, isl], start=True,
                                 stop=True)
                nc.scalar.activation(out_sb[:, isl], ps, AF.Sigmoid,
                                     bias=cb_sb)
                nc.vector.tensor_scalar(out_sb[:, isl], out_sb[:, isl],
                                        1.0 - 1e-10, 1e-10, ALU.min, ALU.max)
                nc.sync.dma_start(out_d[:, isl], out_sb[:, isl])

    nc.finalize()
    return nc


def make_in_maps(inputs, N, n_cores=N_CORES):
    f = {k: np.ascontiguousarray(np.asarray(v, dtype=np.float32))
         for k, v in inputs.items()}
    k1 = f["bn1_g"] / np.sqrt(f["bn1_v"] + BN_EPS)
    c1 = (f["enc_b1"] - f["bn1_m"]) * k1 + f["bn1_b"]
    k2 = f["bn2_g"] / np.sqrt(f["bn2_v"] + BN_EPS)
    c2 = (f["enc_b2"] - f["bn2_m"]) * k2 + f["bn2_b"]
    R = N // n_cores
    shared = dict(
        w1=f["enc_w1"], k1=k1, c1=c1,
        w2=f["enc_w2"], k2=k2, c2=c2,
        g1w=f["gcn1_w"], g1b=f["gcn1_b"],
        g2w=f["gcn2_w"], g2b=f["gcn2_b"],
        cw=f["cls_w"], cb=f["cls_b"],
    )
    maps = []
    for c in range(n_cores):
        r0, r1 = c * R, (c + 1) * R
        m = dict(shared)
        m["a"] = np.ascontiguousarray(f["adj"][r0:r1])
        m["xT"] = np.ascontiguousarray(f["feature"][r0:r1].T)
        maps.append(m)
    return maps


_NC_CACHE = {}


def run(inputs, trace=False, N=16384, n_cores=N_CORES):
    key = (N, n_cores)
    if key not in _NC_CACHE:
        _NC_CACHE[key] = build_nc(N=N, n_cores=n_cores)
    nc = _NC_CACHE[key]
    in_maps = make_in_maps(inputs, N, n_cores)
    res = run_bass_kernel_spmd(nc, in_maps, core_ids=list(range(n_cores)),
                               trace=trace)
    out = np.concatenate([r["out"].T for r in res.results], axis=0)
    return np.ascontiguousarray(out.astype(np.float32)), res


def kernel(**inputs) -> np.ndarray:
    out, _ = run(inputs, trace=False)
    return out



# revision 12
# speedup vs baseline: 1.4654x; 1.4654x over previous
"""GCN classifier on 8 TRN2 NeuronCores — host-transposed fp8 adjacency.

Row-shards the 16384-node graph across 8 cores (2048 rows each). The host
pre-packs each core's row block of the adjacency as A^T in fp8-e4m3 with the
DoubleRow interleave (j = jt*256 + two*128 + p stored as [jt, p, two, i]),
so the device never transposes or casts A. Three streams of the 32 MiB A^T
block per core:

  1. deg pass — DoubleRow ones-matmuls accumulate row degrees while the
     encoder (X@W1 -> BN/relu -> @W2 -> BN/relu -> @G1W) runs on the PE.
     XW1 is cast to fp8 and AllGathered in two chunks (cc1A mid-encoder,
     cc1B after the tiny dis AllGather) so the collective channel stays
     busy behind the deg stream. dis = deg^-0.5.
  2. agg pass 1 — y1 = (A @ (dis_j*xw1))^T in two i-rounds of fp8 DoubleRow
     matmuls (4 PSUM banks each). Round 0 also scales S1 tiles by
     SC1*dis_j into a persistent fp8 buffer that round 1 reuses; its
     j-order consumes the cc1A chunk first. Each round's epilogue applies
     dis_i/SC1 + bias + relu, computes XW2 pre-scaled by SC1*SC2X*dis_i,
     and AllGathers it in fp8 (cc2A overlaps round 1).
  3. agg pass 2 — y2 = (A @ s2)^T with the pre-scaled fp8 S2 read directly,
     two phases keyed to the cc2A/cc2B halves, fused classifier + sigmoid
     tail.

Self-contained: hardcodes shapes for nn_GCNClassifer_6786048328674.
"""

import sys

sys.path.insert(0, "/opt/trn_rl_repo")

from contextlib import ExitStack

import numpy as np
import ml_dtypes

import concourse.bass as bass
from concourse import bacc
import concourse.mybir as mybir
from concourse.tile import TileContext, add_dep_helper
from concourse.bass_utils import run_bass_kernel_spmd
from concourse.masks import make_identity

F32 = mybir.dt.float32
BF16 = mybir.dt.bfloat16
F8 = mybir.dt.float8e4
NP_F8 = ml_dtypes.float8_e4m3
NP_BF16 = ml_dtypes.bfloat16
AF = mybir.ActivationFunctionType
ALU = mybir.AluOpType
DR = mybir.MatmulPerfMode.DoubleRow

BN_EPS = 1e-5
N_CORES = 8
P = 128
SC1 = 256.0          # fp8 pre-quant scale for S1 = SC1*dis_j*xw1
SC2X = 4.0           # extra scale for S2 = SC1*SC2X*dis_j*xw2


def build_nc(N=16384, F=1024, D1=512, E=256, H=256, G=128, C=10,
             n_cores=N_CORES):
    R = N // n_cores            # rows (nodes) per core
    RH = R // 2                 # rows per i-round
    IC = R // 512               # 512-wide i chunks (4)
    IB = R // P                 # 128-wide i blocks (16)
    JT = N // 256               # 256-wide j tiles (64)
    NG = JT // 2                # 2-jt groups per A stream (32)
    CCR = RH // 512             # 512-wide i chunks per round (2)
    NO = N // P                 # 128-wide o blocks, all nodes (128)
    GPC = IB // 4               # j-groups per core row-range (4)
    GPH = IB // 8               # j-groups per (core, xw1 chunk) (2)

    nc = bacc.Bacc(num_devices=n_cores)

    # ---- I/O -------------------------------------------------------------
    # a: A^T row block in fp8, split into i-halves: storage row
    # r = ih*N + jt*256 + p*2 + two holds logical column j = jt*256 +
    # two*128 + p of this core's A rows [ih*RH, (ih+1)*RH).
    a_d = nc.declare_dram_parameter("a", [2 * N, RH], F8, isOutput=False)
    xT_d = nc.declare_dram_parameter("xT", [F, R], BF16, isOutput=False)
    w1_d = nc.declare_dram_parameter("w1", [F, D1], BF16, isOutput=False)
    k1_d = nc.declare_dram_parameter("k1", [D1], F32, isOutput=False)
    c1_d = nc.declare_dram_parameter("c1", [D1], F32, isOutput=False)
    w2_d = nc.declare_dram_parameter("w2", [D1, E], BF16, isOutput=False)
    k2_d = nc.declare_dram_parameter("k2", [E], F32, isOutput=False)
    c2_d = nc.declare_dram_parameter("c2", [E], F32, isOutput=False)
    g1w_d = nc.declare_dram_parameter("g1w", [E, H], BF16, isOutput=False)
    g1b_d = nc.declare_dram_parameter("g1b", [H], F32, isOutput=False)
    g2w_d = nc.declare_dram_parameter("g2w", [H, G], BF16, isOutput=False)
    g2b_d = nc.declare_dram_parameter("g2b", [G], F32, isOutput=False)
    cw_d = nc.declare_dram_parameter("cw", [G, C], BF16, isOutput=False)
    cb_d = nc.declare_dram_parameter("cb", [C], F32, isOutput=False)
    out_d = nc.declare_dram_parameter("out", [C, R], F32, isOutput=True)

    # ---- collective DRAM tensors (Shared declared smallest-first) --------
    dis_loc = nc.dram_tensor("dis_loc", [1, R], F32)
    DISg = nc.dram_tensor("DISg", [n_cores, R], F32, addr_space="Shared")
    s2a_loc = nc.dram_tensor("s2a_loc", [R // 2, G], F8)
    S2gA = nc.dram_tensor("S2gA", [N // 2, G], F8, addr_space="Shared")
    s2b_loc = nc.dram_tensor("s2b_loc", [R // 2, G], F8)
    S2gB = nc.dram_tensor("S2gB", [N // 2, G], F8, addr_space="Shared")
    xw1a_loc = nc.dram_tensor("xw1a_loc", [R // 2, H], F8)
    S1gA = nc.dram_tensor("S1gA", [N // 2, H], F8, addr_space="Shared")
    xw1b_loc = nc.dram_tensor("xw1b_loc", [R // 2, H], F8)
    S1gB = nc.dram_tensor("S1gB", [N // 2, H], F8, addr_space="Shared")
    groups = [list(range(n_cores))]

    with TileContext(nc) as tc, ExitStack() as ctx:
        wpool = ctx.enter_context(tc.tile_pool(name="wpool", bufs=1))
        io_pool = ctx.enter_context(tc.tile_pool(name="io", bufs=3))
        psum = ctx.enter_context(tc.tile_pool(name="psum", bufs=1, space="PSUM"))
        _psn = [0]

        def ps_tile(shape, dtype, banks=range(8), name=None):
            tag = "b%d" % (list(banks)[_psn[0] % len(list(banks))])
            _psn[0] += 1
            return psum.tile(shape, dtype, tag=tag, name=name or f"ps{_psn[0]}")

        # ---- constants / weights in SBUF --------------------------------
        idb = wpool.tile([P, P], BF16)
        make_identity(nc, idb)
        idf = wpool.tile([P, P], F32)
        make_identity(nc, idf)
        # dual-fp8 ldweights needs 16B-aligned weight strides: keep 16 cols
        ones2 = wpool.tile([P, 2, 16], F8)
        nc.vector.memset(ones2, 1.0)

        def load_vec(d, n, nm):
            t = wpool.tile([P, n // P], F32, tag=nm, name=nm)
            nc.gpsimd.dma_start(t, d.ap().rearrange("(o p) -> p o", p=P))
            return t

        w1_sb = wpool.tile([P, F // P, D1], BF16)
        nc.gpsimd.dma_start(w1_sb, w1_d.ap().rearrange("(ko p) m -> p ko m", p=P))
        k1_sb = load_vec(k1_d, D1, "k1v")
        c1_sb = load_vec(c1_d, D1, "c1v")
        w2_sb = wpool.tile([P, D1 // P, E], BF16, name="w2_sb")
        nc.gpsimd.dma_start(w2_sb, w2_d.ap().rearrange("(ko p) m -> p ko m", p=P))
        g1w_sb = wpool.tile([P, E // P, H], BF16, name="g1w_sb")
        nc.gpsimd.dma_start(g1w_sb, g1w_d.ap().rearrange("(ko p) m -> p ko m", p=P))
        g2w_sb = wpool.tile([P, H // P, G], BF16, name="g2w_sb")
        nc.gpsimd.dma_start(g2w_sb, g2w_d.ap().rearrange("(ko p) m -> p ko m", p=P))
        cw_sb = wpool.tile([G, C], BF16, name="cw_sb")
        nc.gpsimd.dma_start(cw_sb, cw_d[:, :])
        k2_sb = load_vec(k2_d, E, "k2v")
        c2_sb = load_vec(c2_d, E, "c2v")
        g1b_sb = load_vec(g1b_d, H, "g1bv")
        g2b_sb = load_vec(g2b_d, G, "g2bv")
        cb_sb = wpool.tile([C, 1], F32, name="cb_sb")
        nc.gpsimd.dma_start(cb_sb, cb_d.ap().rearrange("(c o) -> c o", o=1))

        # ---- persistent activations (feature-major) ----------------------
        h1_sb = wpool.tile([P, D1 // P, R], BF16, tag="tagA")   # 2 MiB
        h2_sb = wpool.tile([P, E // P, R], BF16, tag="tagB")    # 1 MiB
        xw1f_sb = wpool.tile([P, H // P, R], BF16, tag="tagC")  # 1 MiB
        h3_sb = wpool.tile([P, H // P, R], BF16, tag="tagD")    # 1 MiB
        h4_sb = wpool.tile([P, R], BF16, tag="tagE")            # 0.5 MiB
        xw2s_sb = wpool.tile([P, R], BF16, tag="tagF")          # 0.5 MiB
        s1s_all = wpool.tile([P, NO, H], F8, tag="tagG")        # 4 MiB
        out_sb = wpool.tile([C, R], F32)
        dis_bc = wpool.tile([P, R], F32)        # raw dis_i broadcast
        dis_all = wpool.tile([P, NO], F32)      # SC1 * dis_j, all nodes
        dmy = wpool.tile([1, R], F32)

        # A^T access: [ih, p, jt, two, i'] (i' local to the i-half)
        a_r = a_d.ap().rearrange("(ih jt p two) i -> ih p jt two i",
                                 ih=2, p=P, two=2)
        xT_r = xT_d.ap().rearrange("(ko p) i -> p ko i", p=P)
        S1A_r = S1gA.ap().rearrange("(o p) n -> p o n", p=P)
        S1B_r = S1gB.ap().rearrange("(o p) n -> p o n", p=P)
        S2A_r = S2gA.ap().rearrange("(o p) n -> p o n", p=P)
        S2B_r = S2gB.ap().rearrange("(o p) n -> p o n", p=P)

        # ===== encoder strips ============================================
        def enc_strip(s):
            isl = bass.ts(s, 512)
            xs = io_pool.tile([P, F // P, 512], BF16, tag="xs", bufs=2,
                              name=f"xs{s}")
            (nc.sync if s % 2 == 0 else nc.scalar).dma_start(
                xs, xT_r[:, :, isl])
            for m in range(D1 // P):
                ps = ps_tile([P, 512], F32, banks=range(4))
                for k in range(F // P):
                    nc.tensor.matmul(ps, w1_sb[:, k, bass.ts(m, P)], xs[:, k],
                                     start=(k == 0), stop=(k == F // P - 1))
                nc.scalar.activation(h1_sb[:, m, isl], ps, AF.Relu,
                                     bias=c1_sb[:, m:m + 1],
                                     scale=k1_sb[:, m:m + 1])
            for m in range(E // P):
                ps = ps_tile([P, 512], F32, banks=range(4))
                for k in range(D1 // P):
                    nc.tensor.matmul(ps, w2_sb[:, k, bass.ts(m, P)],
                                     h1_sb[:, k, isl],
                                     start=(k == 0), stop=(k == D1 // P - 1))
                nc.scalar.activation(h2_sb[:, m, isl], ps, AF.Relu,
                                     bias=c2_sb[:, m:m + 1],
                                     scale=k2_sb[:, m:m + 1])
            for m in range(H // P):
                ps = ps_tile([P, 512], F32, banks=range(4))
                for k in range(E // P):
                    nc.tensor.matmul(ps, g1w_sb[:, k, bass.ts(m, P)],
                                     h2_sb[:, k, isl],
                                     start=(k == 0), stop=(k == E // P - 1))
                nc.vector.tensor_copy(xw1f_sb[:, m, isl], ps)

        # xw1 chunk -> natural fp8 -> local DRAM (gathered as cc1A/cc1B)
        def xw1_chunk_out(ch):
            loc = xw1a_loc if ch == 0 else xw1b_loc
            for itl in range(IB // 2):
                it = (IB // 2) * ch + itl
                pst = ps_tile([P, H], BF16, banks=range(4))
                for m in range(H // P):
                    nc.tensor.transpose(pst[:, bass.ts(m, P)],
                                        xw1f_sb[:, m, bass.ts(it, P)], idb)
                snat = io_pool.tile([P, H], F8, tag="s1nat")
                nc.scalar.activation(snat, pst, AF.Copy)
                nc.gpsimd.dma_start(loc[bass.ts(itl, P), :], snat)

        # ===== deg pass: stream A^T, accumulate row degrees ==============
        dps = [psum.tile([16, 512], F32, tag=f"b{4 + i}", name=f"degps{i}")
               for i in range(IC)]

        def deg_group(g):
            for ih in range(2):
                att = io_pool.tile([P, 2, 2, RH], F8, tag="a_in", bufs=6,
                                   name=f"dat{g}_{ih}")
                eng = nc.sync if (g + ih) % 2 == 0 else nc.scalar
                eng.dma_start(att, a_r[ih, :, 2 * g:2 * g + 2, :, :])
                for t in range(2):
                    for cc in range(CCR):
                        nc.tensor.matmul(
                            dps[ih * CCR + cc], ones2,
                            att[:, t, :, bass.ts(cc, 512)],
                            start=(g == 0 and t == 0),
                            stop=(g == NG - 1 and t == 1), perf_mode=DR)

        # interleave encoder strips with the head of the deg stream;
        # xw1 fp8 chunks go out as soon as their strips complete
        cc1a = None
        for s in range(IC):
            enc_strip(s)
            deg_group(s)
            if s == IC // 2 - 1:
                xw1_chunk_out(0)
                cc1a = nc.gpsimd.collective_compute(
                    "AllGather", ALU.bypass, replica_groups=groups,
                    ins=[xw1a_loc[:, :].opt()], outs=[S1gA[:, :].opt()])
            if s == IC - 1:
                xw1_chunk_out(1)

        for g in range(IC, NG):
            deg_group(g)

        # ---- dis = deg^-0.5; tiny AllGather ordered before cc1B ---------
        for i in range(IC):
            nc.vector.tensor_copy(dmy[:, bass.ts(i, 512)], dps[i][0:1, :])
        nc.vector.reciprocal(dmy, dmy)
        nc.scalar.activation(dmy, dmy, AF.Sqrt)
        nc.sync.dma_start(dis_loc[0:1, :], dmy)
        ccd = nc.gpsimd.collective_compute(
            "AllGather", ALU.bypass, replica_groups=groups,
            ins=[dis_loc[:, :].opt()], outs=[DISg[:, :].opt()])
        nc.sync.dma_start(dis_bc, dis_loc[0:1, :].to_broadcast([P, R]))
        dis_nat = io_pool.tile([NO, P], F32, tag="disn")
        dd = nc.gpsimd.dma_start(
            dis_nat, DISg.ap().rearrange("c (o2 pb) -> (c o2) pb", pb=P))
        add_dep_helper(dd.ins, ccd.ins, reason="dis read after AG")
        cc1b = nc.gpsimd.collective_compute(
            "AllGather", ALU.bypass, replica_groups=groups,
            ins=[xw1b_loc[:, :].opt()], outs=[S1gB[:, :].opt()])
        ps_dis = psum.tile([P, NO], F32, tag="b4", name="ps_dis")
        nc.tensor.transpose(ps_dis, dis_nat, idf[0:NO, 0:NO])
        nc.vector.tensor_scalar_mul(dis_all, ps_dis, SC1)

        # ===== agg pass 1: y1^T = S1^T A^T, two i-rounds =================
        NM = H // P  # 2
        ps_y = [[[psum.tile([P, 512], F32, tag=f"b{4 * r + CCR * m + cc}",
                            name=f"ps_y{r}_{m}_{cc}")
                  for cc in range(CCR)] for m in range(NM)] for r in range(2)]
        # round-0 j-order: groups covered by cc1A first, then cc1B
        gs0 = ([g for g in range(NG) if g % GPC < GPH]
               + [g for g in range(NG) if g % GPC >= GPH])

        def p2_matmuls(r, g, att, pos, npos):
            for t in range(2):
                for m in range(NM):
                    for cc in range(CCR):
                        nc.tensor.matmul(
                            ps_y[r][m][cc],
                            s1s_all[:, 4 * g + 2 * t:4 * g + 2 * t + 2,
                                    bass.ts(m, P)],
                            att[:, t, :, bass.ts(cc, 512)],
                            start=(pos == 0 and t == 0),
                            stop=(pos == npos - 1 and t == 1), perf_mode=DR)

        def p2_round0(pos, g):
            c, k = g // GPC, g % GPC
            att = io_pool.tile([P, 2, 2, RH], F8, tag="a_h", bufs=6,
                               name=f"a0_{g}")
            eng = nc.sync if pos % 2 == 0 else nc.scalar
            eng.dma_start(att, a_r[0, :, 2 * g:2 * g + 2, :, :])
            s1t = io_pool.tile([P, 4, H], F8, tag="s1t", bufs=3,
                               name=f"s1t{g}")
            eng2 = nc.scalar if pos % 2 == 0 else nc.sync
            if k < GPH:
                d2 = eng2.dma_start(
                    s1t, S1A_r[:, (IB // 2) * c + 4 * k:
                               (IB // 2) * c + 4 * k + 4, :])
                add_dep_helper(d2.ins, cc1a.ins, reason="S1A read after AG")
            else:
                d2 = eng2.dma_start(
                    s1t, S1B_r[:, (IB // 2) * c + 4 * (k - GPH):
                               (IB // 2) * c + 4 * (k - GPH) + 4, :])
                add_dep_helper(d2.ins, cc1b.ins, reason="S1B read after AG")
            for u in range(4):
                nc.vector.tensor_scalar_mul(
                    s1s_all[:, 4 * g + u], s1t[:, u],
                    dis_all[:, 4 * g + u:4 * g + u + 1])
            p2_matmuls(0, g, att, pos, NG)

        def p2_round1(pos, g):
            att = io_pool.tile([P, 2, 2, RH], F8, tag="a_h", bufs=6,
                               name=f"a1_{g}")
            eng = nc.sync if pos % 2 == 0 else nc.scalar
            eng.dma_start(att, a_r[1, :, 2 * g:2 * g + 2, :, :])
            p2_matmuls(1, g, att, pos, NG)

        # epilogue for i-round r: h3, xw2 (pre-scaled fp8), gather half
        def p2_epilogue(r):
            loc = s2a_loc if r == 0 else s2b_loc
            for cc in range(CCR):
                i = CCR * r + cc
                isl = bass.ts(i, 512)
                for m in range(NM):
                    tt = io_pool.tile([P, 512], F32, tag="ep",
                                      name=f"tt{m}_{i}")
                    nc.vector.tensor_tensor(tt, ps_y[r][m][cc],
                                            dis_bc[:, isl], ALU.mult)
                    nc.scalar.activation(h3_sb[:, m, isl], tt, AF.Relu,
                                         bias=g1b_sb[:, m:m + 1],
                                         scale=1.0 / SC1)
                ps_x = psum.tile([P, 512], F32, tag=f"b{4 * r + cc}",
                                 name=f"ps_x{i}")
                for k in range(H // P):
                    nc.tensor.matmul(ps_x, g2w_sb[:, k, :], h3_sb[:, k, isl],
                                     start=(k == 0), stop=(k == H // P - 1))
                nc.vector.tensor_tensor(xw2s_sb[:, isl], ps_x,
                                        dis_bc[:, isl], ALU.mult)
                for itl in range(4):
                    it = 4 * i + itl
                    pst2 = psum.tile([P, G], BF16, tag=f"b{4 * r + CCR + cc}",
                                     name=f"pst2_{it}")
                    nc.tensor.transpose(pst2, xw2s_sb[:, bass.ts(it, P)], idb)
                    snat2 = io_pool.tile([P, G], F8, tag="s2nat",
                                         name=f"sn2_{it}")
                    nc.scalar.activation(snat2, pst2, AF.Copy,
                                         scale=SC1 * SC2X)
                    nc.gpsimd.dma_start(
                        loc[bass.ts(4 * cc + itl, P), :], snat2)
            return nc.gpsimd.collective_compute(
                "AllGather", ALU.bypass, replica_groups=groups,
                ins=[loc[:, :].opt()],
                outs=[(S2gA if r == 0 else S2gB)[:, :].opt()])

        for pos, g in enumerate(gs0):
            p2_round0(pos, g)
        for pos in range(2):
            p2_round1(pos, pos)
        cc2a = p2_epilogue(0)
        for pos in range(2, NG):
            p2_round1(pos, pos)
        cc2b = p2_epilogue(1)

        # ===== agg pass 2: y2^T = S2^T A^T (two phases) ==================
        ps_z = [psum.tile([P, 512], F32, tag=f"b{i}", name=f"ps_z{i}")
                for i in range(IC)]
        JPC = R // 256          # j-tiles per core row-range (8)
        JPH = JPC // 2          # j-tiles per phase (4)
        OB = 2 * JPH            # 128-row o-blocks per (core, phase)
        for phase in range(2):
            S2_r = S2A_r if phase == 0 else S2B_r
            cc2p = cc2a if phase == 0 else cc2b
            for c in range(n_cores):
                atts = []
                for hf in range(JPH // 2):
                    jt0 = JPC * c + JPH * phase + 2 * hf
                    pair = []
                    for ih in range(2):
                        att = io_pool.tile([P, 2, 2, RH], F8, tag="a_in",
                                           bufs=6,
                                           name=f"pat{phase}_{c}_{hf}_{ih}")
                        eng = nc.sync if (hf + ih) % 2 == 0 else nc.scalar
                        eng.dma_start(att, a_r[ih, :, jt0:jt0 + 2, :, :])
                        pair.append(att)
                    atts.append(pair)
                s2t = io_pool.tile([P, OB, G], F8, tag="s2t", bufs=3,
                                   name=f"s2t{phase}_{c}")
                eng2 = nc.scalar if c % 2 == 0 else nc.sync
                d2 = eng2.dma_start(s2t, S2_r[:, OB * c:OB * (c + 1), :])
                add_dep_helper(d2.ins, cc2p.ins, reason="S2 read after AG")
                for jl in range(JPH):
                    pair = atts[jl // 2]
                    tl = jl % 2
                    for ih in range(2):
                        for cc in range(CCR):
                            nc.tensor.matmul(
                                ps_z[ih * CCR + cc],
                                s2t[:, 2 * jl:2 * jl + 2, :],
                                pair[ih][:, tl, :, bass.ts(cc, 512)],
                                start=(phase == 0 and c == 0 and jl == 0),
                                stop=(phase == 1 and c == n_cores - 1
                                      and jl == JPH - 1),
                                perf_mode=DR)

        # ==== fused tail: dis_i scaling -> relu -> classifier -> out =====
        for i in range(IC):
            isl = bass.ts(i, 512)
            tt = io_pool.tile([P, 512], F32, tag="ep", name=f"ttz{i}")
            nc.vector.tensor_tensor(tt, ps_z[i], dis_bc[:, isl], ALU.mult)
            nc.scalar.activation(h4_sb[:, isl], tt, AF.Relu,
                                 bias=g2b_sb[:, 0:1],
                                 scale=1.0 / (SC1 * SC2X))
            ps_c = psum.tile([C, 512], F32, tag=f"b{4 + (i % 2)}",
                             name=f"ps_c{i}")
            nc.tensor.matmul(ps_c, cw_sb, h4_sb[:, isl], start=True, stop=True)
            nc.scalar.activation(out_sb[:, isl], ps_c, AF.Sigmoid, bias=cb_sb)
            nc.sync.dma_start(out_d[:, isl], out_sb[:, isl])

    nc.finalize()
    return nc


def make_in_maps(inputs, N, n_cores=N_CORES):
    f = {k: np.ascontiguousarray(np.asarray(v, dtype=np.float32))
         for k, v in inputs.items()}
    k1 = f["bn1_g"] / np.sqrt(f["bn1_v"] + BN_EPS)
    c1 = (f["enc_b1"] - f["bn1_m"]) * k1 + f["bn1_b"]
    k2 = f["bn2_g"] / np.sqrt(f["bn2_v"] + BN_EPS)
    c2 = (f["enc_b2"] - f["bn2_m"]) * k2 + f["bn2_b"]
    R = N // n_cores
    shared = dict(
        w1=f["enc_w1"].astype(NP_BF16), k1=k1, c1=c1,
        w2=f["enc_w2"].astype(NP_BF16), k2=k2, c2=c2,
        g1w=f["gcn1_w"].astype(NP_BF16), g1b=f["gcn1_b"],
        g2w=f["gcn2_w"].astype(NP_BF16), g2b=f["gcn2_b"],
        cw=f["cls_w"].astype(NP_BF16), cb=f["cls_b"],
    )
    maps = []
    for c in range(n_cores):
        r0, r1 = c * R, (c + 1) * R
        m = dict(shared)
        # fp8 cast then byte-transpose into [ih, jt, p, two, i'] (cheaper
        # than casting a strided view); logical j = jt*256 + two*128 + p.
        a8 = f["adj"][r0:r1].astype(NP_F8)
        a8 = np.ascontiguousarray(
            a8.reshape(2, R // 2, N // 256, 2, P).transpose(0, 2, 4, 3, 1))
        m["a"] = a8.reshape(2 * N, R // 2)
        m["xT"] = np.ascontiguousarray(
            f["feature"][r0:r1].astype(NP_BF16).T)
        maps.append(m)
    return maps


_NC_CACHE = {}


def run(inputs, trace=False, N=16384, n_cores=N_CORES):
    key = (N, n_cores)
    if key not in _NC_CACHE:
        _NC_CACHE[key] = build_nc(N=N, n_cores=n_cores)
    nc = _NC_CACHE[key]
    in_maps = make_in_maps(inputs, N, n_cores)
    res = run_bass_kernel_spmd(nc, in_maps, core_ids=list(range(n_cores)),
                               trace=trace)
    out = np.concatenate([r["out"].T for r in res.results], axis=0)
    return np.ascontiguousarray(out.astype(np.float32)), res


def kernel(**inputs) -> np.ndarray:
    out, _ = run(inputs, trace=False)
    return out


# revision 17
# speedup vs baseline: 1.6005x; 1.0921x over previous
"""GCN classifier on 8 TRN2 NeuronCores — host-transposed fp8 adjacency.

Row-shards the 16384-node graph across 8 cores (2048 rows each). The host
pre-packs each core's row block of the adjacency as A^T in fp8-e4m3 with the
DoubleRow interleave (j = jt*256 + two*128 + p stored as [jt, p, two, i]),
so the device never transposes or casts A. Three streams of the 32 MiB A^T
block per core:

  1. deg pass — DoubleRow ones-matmuls accumulate row degrees while the
     encoder (X@W1 -> BN/relu -> @W2 -> BN/relu -> @G1W) runs on the PE.
     XW1 is cast to fp8 and AllGathered in two chunks (cc1A mid-encoder,
     cc1B after the tiny dis AllGather) so the collective channel stays
     busy behind the deg stream. dis = deg^-0.5.
  2. agg pass 1 — y1 = (A @ (dis_j*xw1))^T in two i-rounds of fp8 DoubleRow
     matmuls (4 PSUM banks each). Round 0 also scales S1 tiles by
     SC1*dis_j into a persistent fp8 buffer that round 1 reuses; its
     j-order consumes the cc1A chunk first. Each round's epilogue applies
     dis_i/SC1 + bias + relu, computes XW2 pre-scaled by SC1*SC2X*dis_i,
     and AllGathers it in fp8 (cc2A overlaps round 1).
  3. agg pass 2 — y2 = (A @ s2)^T with the pre-scaled fp8 S2 read directly,
     two phases keyed to the cc2A/cc2B halves, fused classifier + sigmoid
     tail.

Self-contained: hardcodes shapes for nn_GCNClassifer_6786048328674.
"""

import sys

sys.path.insert(0, "/opt/trn_rl_repo")

from contextlib import ExitStack

import numpy as np
import ml_dtypes

import concourse.bass as bass
from concourse import bacc
import concourse.mybir as mybir
from concourse.tile import TileContext, add_dep_helper
from concourse.bass_utils import run_bass_kernel_spmd
from concourse.masks import make_identity

F32 = mybir.dt.float32
BF16 = mybir.dt.bfloat16
F8 = mybir.dt.float8e4
NP_F8 = ml_dtypes.float8_e4m3
NP_BF16 = ml_dtypes.bfloat16
AF = mybir.ActivationFunctionType
ALU = mybir.AluOpType
DR = mybir.MatmulPerfMode.DoubleRow

BN_EPS = 1e-5
N_CORES = 8
P = 128
SC1 = 256.0          # fp8 pre-quant scale for S1 = SC1*dis_j*xw1
SC2X = 4.0           # extra scale for S2 = SC1*SC2X*dis_j*xw2


def build_nc(N=16384, F=1024, D1=512, E=256, H=256, G=128, C=10,
             n_cores=N_CORES, stop=None, no_deg_mm=False, no_coll=False,
             no_enc=False):
    R = N // n_cores            # rows (nodes) per core
    RH = R // 2                 # rows per i-round
    IC = R // 512               # 512-wide i chunks (4)
    IB = R // P                 # 128-wide i blocks (16)
    JT = N // 256               # 256-wide j tiles (64)
    NG = JT // 2                # 2-jt groups per A stream (32)
    CCR = RH // 512             # 512-wide i chunks per round (2)
    NO = N // P                 # 128-wide o blocks, all nodes (128)
    GPC = IB // 4               # j-groups per core row-range (4)
    GPH = IB // 8               # j-groups per (core, xw1 chunk) (2)

    nc = bacc.Bacc(num_devices=n_cores)

    # ---- I/O -------------------------------------------------------------
    # a: A^T row block in fp8, split into i-halves: storage row
    # r = ih*N + jt*256 + p*2 + two holds logical column j = jt*256 +
    # two*128 + p of this core's A rows [ih*RH, (ih+1)*RH).
    a_d = nc.declare_dram_parameter("a", [2 * N, RH], F8, isOutput=False)
    xT_d = nc.declare_dram_parameter("xT", [F, R], BF16, isOutput=False)
    w1_d = nc.declare_dram_parameter("w1", [F, D1], BF16, isOutput=False)
    k1_d = nc.declare_dram_parameter("k1", [D1], F32, isOutput=False)
    c1_d = nc.declare_dram_parameter("c1", [D1], F32, isOutput=False)
    w2_d = nc.declare_dram_parameter("w2", [D1, E], BF16, isOutput=False)
    k2_d = nc.declare_dram_parameter("k2", [E], F32, isOutput=False)
    c2_d = nc.declare_dram_parameter("c2", [E], F32, isOutput=False)
    g1w_d = nc.declare_dram_parameter("g1w", [E, H], BF16, isOutput=False)
    g1b_d = nc.declare_dram_parameter("g1b", [H], F32, isOutput=False)
    g2w_d = nc.declare_dram_parameter("g2w", [H, G], BF16, isOutput=False)
    g2b_d = nc.declare_dram_parameter("g2b", [G], F32, isOutput=False)
    cw_d = nc.declare_dram_parameter("cw", [G, C], BF16, isOutput=False)
    cb_d = nc.declare_dram_parameter("cb", [C], F32, isOutput=False)
    out_d = nc.declare_dram_parameter("out", [C, R], F32, isOutput=True)

    # ---- collective DRAM tensors (Shared declared smallest-first) --------
    dis_loc = nc.dram_tensor("dis_loc", [1, R], F32)
    s2a_loc = nc.dram_tensor("s2a_loc", [R // 2, G], F8)
    S2gA = nc.dram_tensor("S2gA", [N // 2, G], F8, addr_space="Shared")
    s2b_loc = nc.dram_tensor("s2b_loc", [R // 2, G], F8)
    S2gB = nc.dram_tensor("S2gB", [N // 2, G], F8, addr_space="Shared")
    xw1_loc = nc.dram_tensor("xw1_loc", [R, H], F8)
    S1g = nc.dram_tensor("S1g", [N, H], F8, addr_space="Shared")
    groups = [list(range(n_cores))]

    with TileContext(nc) as tc, ExitStack() as ctx:
        wpool = ctx.enter_context(tc.tile_pool(name="wpool", bufs=1))
        io_pool = ctx.enter_context(tc.tile_pool(name="io", bufs=3))
        psum = ctx.enter_context(tc.tile_pool(name="psum", bufs=1, space="PSUM"))
        _psn = [0]

        def ps_tile(shape, dtype, banks=range(8), name=None):
            tag = "b%d" % (list(banks)[_psn[0] % len(list(banks))])
            _psn[0] += 1
            return psum.tile(shape, dtype, tag=tag, name=name or f"ps{_psn[0]}")

        # ---- constants / weights in SBUF --------------------------------
        idb = wpool.tile([P, P], BF16)
        make_identity(nc, idb)
        # dual-fp8 ldweights needs 16B-aligned weight strides: keep 16 cols
        ones2 = wpool.tile([P, 2, 16], F8)
        nc.vector.memset(ones2, 1.0)

        def load_vec(d, n, nm):
            t = wpool.tile([P, n // P], F32, tag=nm, name=nm)
            nc.gpsimd.dma_start(t, d.ap().rearrange("(o p) -> p o", p=P))
            return t

        w1_sb = wpool.tile([P, F // P, D1], BF16)
        nc.gpsimd.dma_start(w1_sb, w1_d.ap().rearrange("(ko p) m -> p ko m", p=P))
        k1_sb = load_vec(k1_d, D1, "k1v")
        c1_sb = load_vec(c1_d, D1, "c1v")
        w2_sb = wpool.tile([P, D1 // P, E], BF16, name="w2_sb")
        nc.gpsimd.dma_start(w2_sb, w2_d.ap().rearrange("(ko p) m -> p ko m", p=P))
        g1w_sb = wpool.tile([P, E // P, H], BF16, name="g1w_sb")
        nc.gpsimd.dma_start(g1w_sb, g1w_d.ap().rearrange("(ko p) m -> p ko m", p=P))
        g2w_sb = wpool.tile([P, H // P, G], BF16, name="g2w_sb")
        nc.gpsimd.dma_start(g2w_sb, g2w_d.ap().rearrange("(ko p) m -> p ko m", p=P))
        cw_sb = wpool.tile([G, C], BF16, name="cw_sb")
        nc.gpsimd.dma_start(cw_sb, cw_d[:, :])
        k2_sb = load_vec(k2_d, E, "k2v")
        c2_sb = load_vec(c2_d, E, "c2v")
        g1b_sb = load_vec(g1b_d, H, "g1bv")
        g2b_sb = load_vec(g2b_d, G, "g2bv")
        cb_sb = wpool.tile([C, 1], F32, name="cb_sb")
        nc.gpsimd.dma_start(cb_sb, cb_d.ap().rearrange("(c o) -> c o", o=1))

        # ---- persistent activations (feature-major) ----------------------
        h1_sb = wpool.tile([P, D1 // P, R], BF16, tag="tagA")   # 2 MiB
        h2_sb = wpool.tile([P, E // P, R], BF16, tag="tagB")    # 1 MiB
        xw1f_sb = wpool.tile([P, H // P, R], BF16, tag="tagC")  # 1 MiB
        h3_sb = wpool.tile([P, H // P, R], BF16, tag="tagD")    # 1 MiB
        h4_sb = wpool.tile([P, R], BF16, tag="tagE")            # 0.5 MiB
        xw2s_sb = wpool.tile([P, R], BF16, tag="tagF")          # 0.5 MiB
        s1s_all = wpool.tile([P, NO, H], F8, tag="tagG")        # 4 MiB
        out_sb = wpool.tile([C, R], F32)
        dis_bc = wpool.tile([P, R], F32)        # raw dis_i broadcast
        xw1s_sb = wpool.tile([P, H // P, R], BF16, tag="tagH")  # 1 MiB
        dmy = wpool.tile([1, R], F32)

        # A^T access: [ih, p, jt, two, i'] (i' local to the i-half)
        a_r = a_d.ap().rearrange("(ih jt p two) i -> ih p jt two i",
                                 ih=2, p=P, two=2)
        xT_r = xT_d.ap().rearrange("(ko p) i -> p ko i", p=P)
        S1_r = S1g.ap().rearrange("(o p) n -> p o n", p=P)
        S2A_r = S2gA.ap().rearrange("(o p) n -> p o n", p=P)
        S2B_r = S2gB.ap().rearrange("(o p) n -> p o n", p=P)

        # ===== encoder strips ============================================
        def enc_strip(s):
            isl = bass.ts(s, 512)
            xs = io_pool.tile([P, F // P, 512], BF16, tag="xs", bufs=2,
                              name=f"xs{s}")
            (nc.sync if s % 2 == 0 else nc.scalar).dma_start(
                xs, xT_r[:, :, isl])
            if no_enc:
                return
            for m in range(D1 // P):
                ps = ps_tile([P, 512], F32, banks=range(4))
                for k in range(F // P):
                    nc.tensor.matmul(ps, w1_sb[:, k, bass.ts(m, P)], xs[:, k],
                                     start=(k == 0), stop=(k == F // P - 1))
                nc.scalar.activation(h1_sb[:, m, isl], ps, AF.Relu,
                                     bias=c1_sb[:, m:m + 1],
                                     scale=k1_sb[:, m:m + 1])
            for m in range(E // P):
                ps = ps_tile([P, 512], F32, banks=range(4))
                for k in range(D1 // P):
                    nc.tensor.matmul(ps, w2_sb[:, k, bass.ts(m, P)],
                                     h1_sb[:, k, isl],
                                     start=(k == 0), stop=(k == D1 // P - 1))
                nc.scalar.activation(h2_sb[:, m, isl], ps, AF.Relu,
                                     bias=c2_sb[:, m:m + 1],
                                     scale=k2_sb[:, m:m + 1])
            for m in range(H // P):
                ps = ps_tile([P, 512], F32, banks=range(4))
                for k in range(E // P):
                    nc.tensor.matmul(ps, g1w_sb[:, k, bass.ts(m, P)],
                                     h2_sb[:, k, isl],
                                     start=(k == 0), stop=(k == E // P - 1))
                nc.vector.tensor_copy(xw1f_sb[:, m, isl], ps)

        # dis_j-scaled xw1 -> natural fp8 (xSC1) -> local DRAM for cc1
        def xw1_out(it):
            pst = ps_tile([P, H], BF16, banks=range(4))
            for m in range(H // P):
                nc.tensor.transpose(pst[:, bass.ts(m, P)],
                                    xw1s_sb[:, m, bass.ts(it, P)], idb)
            snat = io_pool.tile([P, H], F8, tag="s1nat")
            nc.scalar.activation(snat, pst, AF.Copy, scale=SC1)
            nc.gpsimd.dma_start(xw1_loc[bass.ts(it, P), :], snat)

        # ===== deg pass: stream A^T, accumulate row degrees ==============
        dps = [psum.tile([16, 512], F32, tag=f"b{4 + i}", name=f"degps{i}")
               for i in range(IC)]

        def deg_group(g):
            for ih in range(2):
                att = io_pool.tile([P, 2, 2, RH], F8, tag="a_in", bufs=6,
                                   name=f"dat{g}_{ih}")
                eng = nc.sync if (g + ih) % 2 == 0 else nc.scalar
                eng.dma_start(att, a_r[ih, :, 2 * g:2 * g + 2, :, :])
                for t in range(0 if no_deg_mm else 2):
                    for cc in range(CCR):
                        nc.tensor.matmul(
                            dps[ih * CCR + cc], ones2,
                            att[:, t, :, bass.ts(cc, 512)],
                            start=(g == 0 and t == 0),
                            stop=(g == NG - 1 and t == 1), perf_mode=DR)

        # interleave encoder strips with the head of the deg stream
        for s in range(IC):
            enc_strip(s)
            deg_group(s)

        for g in range(IC, NG):
            deg_group(g)

        # ---- dis = deg^-0.5; tiny AllGather ordered before cc1B ---------
        if no_deg_mm:
            nc.vector.memset(dmy, 1.0)
        else:
            for i in range(IC):
                nc.vector.tensor_copy(dmy[:, bass.ts(i, 512)], dps[i][0:1, :])
        nc.vector.reciprocal(dmy, dmy)
        nc.scalar.activation(dmy, dmy, AF.Sqrt)
        nc.sync.dma_start(dis_loc[0:1, :], dmy)
        nc.sync.dma_start(dis_bc, dis_loc[0:1, :].to_broadcast([P, R]))
        # scale xw1 by own-rows dis (= dis_j post-gather), ship fp8, gather
        for m in range(H // P):
            for i in range(IC):
                isl = bass.ts(i, 512)
                nc.vector.tensor_tensor(xw1s_sb[:, m, isl],
                                        xw1f_sb[:, m, isl],
                                        dis_bc[:, isl], ALU.mult)
        for it in range(IB):
            xw1_out(it)
        if not no_coll:
            cc1 = nc.gpsimd.collective_compute(
                "AllGather", ALU.bypass, replica_groups=groups,
                ins=[xw1_loc[:, :].opt()], outs=[S1g[:, :].opt()])

        if stop == "deg":
            nc.vector.tensor_copy(out_sb[:, 0:NO], dis_bc[0:C, 0:NO])
            nc.sync.dma_start(out_d[:, :], out_sb)

        # ===== agg pass 1: y1^T = S1^T A^T, two i-rounds =================
        NM = H // P  # 2
        ps_y = [[[psum.tile([P, 512], F32, tag=f"b{4 * r + CCR * m + cc}",
                            name=f"ps_y{r}_{m}_{cc}")
                  for cc in range(CCR)] for m in range(NM)] for r in range(2)]


        def p2_matmuls(r, g, att, pos, npos):
            for t in range(2):
                for m in range(NM):
                    for cc in range(CCR):
                        nc.tensor.matmul(
                            ps_y[r][m][cc],
                            s1s_all[:, 4 * g + 2 * t:4 * g + 2 * t + 2,
                                    bass.ts(m, P)],
                            att[:, t, :, bass.ts(cc, 512)],
                            start=(pos == 0 and t == 0),
                            stop=(pos == npos - 1 and t == 1), perf_mode=DR)

        def p2_round0(pos, g):
            att = io_pool.tile([P, 2, 2, RH], F8, tag="a_h", bufs=8,
                               name=f"a0_{g}")
            eng = nc.sync if pos % 2 == 0 else nc.scalar
            eng.dma_start(att, a_r[0, :, 2 * g:2 * g + 2, :, :])
            eng2 = nc.scalar if pos % 2 == 0 else nc.sync
            d2 = eng2.dma_start(s1s_all[:, 4 * g:4 * g + 4, :],
                                S1_r[:, 4 * g:4 * g + 4, :])
            add_dep_helper(d2.ins, cc1.ins, reason="S1 read after AG")
            p2_matmuls(0, g, att, pos, NG)

        def p2_round1(pos, g):
            att = io_pool.tile([P, 2, 2, RH], F8, tag="a_h", bufs=8,
                               name=f"a1_{g}")
            eng = nc.sync if pos % 2 == 0 else nc.scalar
            eng.dma_start(att, a_r[1, :, 2 * g:2 * g + 2, :, :])
            p2_matmuls(1, g, att, pos, NG)

        # epilogue for i-round r: h3, xw2 (pre-scaled fp8), gather half
        def p2_epilogue(r):
            loc = s2a_loc if r == 0 else s2b_loc
            for cc in range(CCR):
                i = CCR * r + cc
                isl = bass.ts(i, 512)
                for m in range(NM):
                    tt = io_pool.tile([P, 512], F32, tag="ep",
                                      name=f"tt{m}_{i}")
                    nc.vector.tensor_tensor(tt, ps_y[r][m][cc],
                                            dis_bc[:, isl], ALU.mult)
                    nc.scalar.activation(h3_sb[:, m, isl], tt, AF.Relu,
                                         bias=g1b_sb[:, m:m + 1],
                                         scale=1.0 / SC1)
                ps_x = psum.tile([P, 512], F32, tag=f"b{4 * r + cc}",
                                 name=f"ps_x{i}")
                for k in range(H // P):
                    nc.tensor.matmul(ps_x, g2w_sb[:, k, :], h3_sb[:, k, isl],
                                     start=(k == 0), stop=(k == H // P - 1))
                nc.vector.tensor_tensor(xw2s_sb[:, isl], ps_x,
                                        dis_bc[:, isl], ALU.mult)
                for itl in range(4):
                    it = 4 * i + itl
                    pst2 = psum.tile([P, G], BF16, tag=f"b{4 * r + CCR + cc}",
                                     name=f"pst2_{it}")
                    nc.tensor.transpose(pst2, xw2s_sb[:, bass.ts(it, P)], idb)
                    snat2 = io_pool.tile([P, G], F8, tag="s2nat",
                                         name=f"sn2_{it}")
                    nc.scalar.activation(snat2, pst2, AF.Copy,
                                         scale=SC1 * SC2X)
                    nc.gpsimd.dma_start(
                        loc[bass.ts(4 * cc + itl, P), :], snat2)
            return nc.gpsimd.collective_compute(
                "AllGather", ALU.bypass, replica_groups=groups,
                ins=[loc[:, :].opt()],
                outs=[(S2gA if r == 0 else S2gB)[:, :].opt()])

        if stop != "deg":
            for pos in range(NG):
                p2_round0(pos, pos)
            for pos in range(2):
                p2_round1(pos, pos)
            cc2a = p2_epilogue(0)
            for pos in range(2, NG):
                p2_round1(pos, pos)
            cc2b = p2_epilogue(1)

        if stop == "p2":
            nc.vector.tensor_copy(out_sb[0:C, :], h3_sb[0:C, 0, :])
            nc.sync.dma_start(out_d[:, :], out_sb)

        # ===== agg pass 2: y2^T = S2^T A^T (two phases) ==================
        ps_z = [psum.tile([P, 512], F32, tag=f"b{i}", name=f"ps_z{i}")
                for i in range(IC)] if stop is None else []
        JPC = R // 256          # j-tiles per core row-range (8)
        JPH = JPC // 2          # j-tiles per phase (4)
        OB = 2 * JPH            # 128-row o-blocks per (core, phase)
        for phase in range(2 if stop is None else 0):
            S2_r = S2A_r if phase == 0 else S2B_r
            cc2p = cc2a if phase == 0 else cc2b
            for c in range(n_cores):
                atts = []
                for hf in range(JPH // 2):
                    jt0 = JPC * c + JPH * phase + 2 * hf
                    pair = []
                    for ih in range(2):
                        att = io_pool.tile([P, 2, 2, RH], F8, tag="a_in",
                                           bufs=6,
                                           name=f"pat{phase}_{c}_{hf}_{ih}")
                        eng = nc.sync if (hf + ih) % 2 == 0 else nc.scalar
                        eng.dma_start(att, a_r[ih, :, jt0:jt0 + 2, :, :])
                        pair.append(att)
                    atts.append(pair)
                s2t = io_pool.tile([P, OB, G], F8, tag="s2t", bufs=3,
                                   name=f"s2t{phase}_{c}")
                eng2 = nc.scalar if c % 2 == 0 else nc.sync
                d2 = eng2.dma_start(s2t, S2_r[:, OB * c:OB * (c + 1), :])
                add_dep_helper(d2.ins, cc2p.ins, reason="S2 read after AG")
                for jl in range(JPH):
                    pair = atts[jl // 2]
                    tl = jl % 2
                    for ih in range(2):
                        for cc in range(CCR):
                            nc.tensor.matmul(
                                ps_z[ih * CCR + cc],
                                s2t[:, 2 * jl:2 * jl + 2, :],
                                pair[ih][:, tl, :, bass.ts(cc, 512)],
                                start=(phase == 0 and c == 0 and jl == 0),
                                stop=(phase == 1 and c == n_cores - 1
                                      and jl == JPH - 1),
                                perf_mode=DR)

        # ==== fused tail: dis_i scaling -> relu -> classifier -> out =====
        for i in range(IC if stop is None else 0):
            isl = bass.ts(i, 512)
            tt = io_pool.tile([P, 512], F32, tag="ep", name=f"ttz{i}")
            nc.vector.tensor_tensor(tt, ps_z[i], dis_bc[:, isl], ALU.mult)
            nc.scalar.activation(h4_sb[:, isl], tt, AF.Relu,
                                 bias=g2b_sb[:, 0:1],
                                 scale=1.0 / (SC1 * SC2X))
            ps_c = psum.tile([C, 512], F32, tag=f"b{4 + (i % 2)}",
                             name=f"ps_c{i}")
            nc.tensor.matmul(ps_c, cw_sb, h4_sb[:, isl], start=True, stop=True)
            nc.scalar.activation(out_sb[:, isl], ps_c, AF.Sigmoid, bias=cb_sb)
            nc.sync.dma_start(out_d[:, isl], out_sb[:, isl])

    nc.finalize()
    return nc


def make_in_maps(inputs, N, n_cores=N_CORES):
    f = {k: np.ascontiguousarray(np.asarray(v, dtype=np.float32))
         for k, v in inputs.items()}
    k1 = f["bn1_g"] / np.sqrt(f["bn1_v"] + BN_EPS)
    c1 = (f["enc_b1"] - f["bn1_m"]) * k1 + f["bn1_b"]
    k2 = f["bn2_g"] / np.sqrt(f["bn2_v"] + BN_EPS)
    c2 = (f["enc_b2"] - f["bn2_m"]) * k2 + f["bn2_b"]
    R = N // n_cores
    shared = dict(
        w1=f["enc_w1"].astype(NP_BF16), k1=k1, c1=c1,
        w2=f["enc_w2"].astype(NP_BF16), k2=k2, c2=c2,
        g1w=f["gcn1_w"].astype(NP_BF16), g1b=f["gcn1_b"],
        g2w=f["gcn2_w"].astype(NP_BF16), g2b=f["gcn2_b"],
        cw=f["cls_w"].astype(NP_BF16), cb=f["cls_b"],
    )
    maps = []
    for c in range(n_cores):
        r0, r1 = c * R, (c + 1) * R
        m = dict(shared)
        # fp8 cast then byte-transpose into [ih, jt, p, two, i'] (cheaper
        # than casting a strided view); logical j = jt*256 + two*128 + p.
        a8 = f["adj"][r0:r1].astype(NP_F8)
        a8 = np.ascontiguousarray(
            a8.reshape(2, R // 2, N // 256, 2, P).transpose(0, 2, 4, 3, 1))
        m["a"] = a8.reshape(2 * N, R // 2)
        m["xT"] = np.ascontiguousarray(
            f["feature"][r0:r1].astype(NP_BF16).T)
        maps.append(m)
    return maps


_NC_CACHE = {}


def run(inputs, trace=False, N=16384, n_cores=N_CORES):
    key = (N, n_cores)
    if key not in _NC_CACHE:
        _NC_CACHE[key] = build_nc(N=N, n_cores=n_cores)
    nc = _NC_CACHE[key]
    in_maps = make_in_maps(inputs, N, n_cores)
    res = run_bass_kernel_spmd(nc, in_maps, core_ids=list(range(n_cores)),
                               trace=trace)
    out = np.concatenate([r["out"].T for r in res.results], axis=0)
    return np.ascontiguousarray(out.astype(np.float32)), res


def kernel(**inputs) -> np.ndarray:
    out, _ = run(inputs, trace=False)
    return out
